# revision 25
# baseline (speedup 1.0000x reference)
"""Multi-head attention (B=2, S=4096, D=512, H=8) on 8 NeuronCores.

Sharding: data-parallel on batch x head-pair-parallel.  Core c handles
batch b = c//4 and heads (2*(c%4), 2*(c%4)+1); the host scatters inputs
and gathers/normalizes outputs.

Per-core kernel (Bass/Tile), fp16 operands, fp32 PSUM accumulate:
  - DMA order tuned for the 8 HW queue groups: wk chunks + x^T block 0
    first, the other weights pre-swizzled host-side into single 1KB-row
    [128,512] DMAs; biases ship as one [1,384] row and are PE-transposed
    to per-partition columns on core.
  - Prologue per 512-wide chunk: K^T/V^T/Q^T projections (weight
    stationary, x^T streaming, bias-add on the otherwise idle Scalar
    engine); V^T tiles are PE-transposed back to natural [k, d] layout
    into 128-wide [V_h | ones | 0] weight slots (the ones column makes
    the E@V matmul emit softmax row sums in PSUM row 64).
  - Attention per 512-wide q chunk / 128-wide k tile:
      S^T tile  = K^T.T @ Q^T   (row-packed K=64 matmul pairs)
      E         = exp(S^T / 8)  (ACT over [128,1536] PSUM chunks, the
                                 1.2GHz-bound critical path: 176
                                 back-to-back EXPs ~= 250us)
      O^T      += V1.T @ E      (PSUM row 64 accumulates row sums)
    E@V batches are emitted TWO batches late so a deferred EV's exp is
    always complete -- the in-order PE queue then never stalls the next
    batch's S^T matmuls behind an exp-blocked EV.
  - Output: unnormalized O^T halves + row-sum rows DMA out as fp16
    [130, S]; the softmax division happens on the host in _gather.

Measured on the 8 axon trn2 cores: ~299.5us HW exec (329us baseline),
rel err ~7.6e-4.  At full clock the exp stream is gapless; occasional
runs see a ~20% lower DVFS clock (~358us) independent of the kernel.
"""

import numpy as np

N_CORES = 8
S_FULL = 4096
D_MODEL = 512
HEAD = 64

_cached = {}


def build_nc(S=S_FULL):
    import concourse.bass as bass
    from concourse import bacc
    import concourse.mybir as mybir
    import concourse.tile as tile
    from concourse.masks import make_identity
    f32 = mybir.dt.float32
    f16 = mybir.dt.float16
    AF = mybir.ActivationFunctionType

    D = D_MODEL
    n_qc = S // 512     # 512-wide query chunks
    n_kc = S // 128     # 128-wide key tiles
    n_dc = D // 128     # 128-wide contraction chunks of D

    nc = bacc.Bacc()

    xT = nc.dram_tensor("xT", [D, S], f16, kind="ExternalInput")
    wqS = nc.dram_tensor("wqS", [128, 512], f16, kind="ExternalInput")
    wkT = nc.dram_tensor("wkT", [D, 128], f16, kind="ExternalInput")
    wvS = nc.dram_tensor("wvS", [128, 512], f16, kind="ExternalInput")
    bkvqT = nc.dram_tensor("bkvqT", [1, 384], f32, kind="ExternalInput")
    outT = nc.dram_tensor("outT", [130, S], f16, kind="ExternalOutput")

    with tile.TileContext(nc) as tc:
        with (
            tc.tile_pool(name="consts", bufs=1) as consts,
            tc.tile_pool(name="persist", bufs=1) as persist,
        ):
            ident = consts.tile([128, 128], f16, name="ident")
            make_identity(nc, ident)
            # Preload the exp table set while DMAs run.
            warm = consts.tile([128, 1], f16, name="warm")
            nc.scalar.activation(warm[:], ident[:, 0:1], AF.Exp, scale=0.125)

            wq_sb = consts.tile([128, n_dc * 128], f16, name="wq_sb")
            wk_sb = consts.tile([128, n_dc * 128], f16, name="wk_sb")
            wv_sb = consts.tile([128, n_dc * 128], f16, name="wv_sb")
            bq_sb = consts.tile([128, 1], f32, name="bq_sb")
            bk_sb = consts.tile([128, 1], f32, name="bk_sb")
            bv_sb = consts.tile([128, 1], f32, name="bv_sb")
            bsb = consts.tile([1, 384], f32, name="bsb")
            ones11 = consts.tile([1, 1], f32, name="ones11")
            nc.vector.memset(ones11[:], 1.0)
            xt = [persist.tile([128, S], f16, name=f"xt{i}") for i in range(n_dc)]
            # bias rows (3 descriptors) + x block 0 + wk first: they land at
            # the head of the 8 HW queue groups (round-robin by emission
            # order) and run in parallel; the rest follows in later rounds.
            # wk split in 4 (earliest arrival), x block 0 in halves; the
            # other weights ship pre-swizzled as single 1KB-row DMAs
            for dc in range(n_dc):
                r = slice(dc * 128, (dc + 1) * 128)
                nc.sync.dma_start(wk_sb[:, dc * 128:(dc + 1) * 128], wkT[r, :])
            for dc in range(n_dc):
                for hh in range(2):
                    nc.sync.dma_start(
                        xt[dc][:, hh * 256:(hh + 1) * 256],
                        xT[dc * 128:(dc + 1) * 128, hh * 256:(hh + 1) * 256],
                    )
            nc.sync.dma_start(bsb[:], bkvqT[:, :])
            nc.sync.dma_start(wv_sb[:], wvS[:, :])
            nc.sync.dma_start(wq_sb[:], wqS[:, :])
            for j in range(1, n_qc):
                cs = slice(j * 512, (j + 1) * 512)
                for dc in range(n_dc):
                    nc.sync.dma_start(xt[dc][:, cs],
                                      xT[dc * 128:(dc + 1) * 128, cs])

            qt = persist.tile([128, S], f16, name="qt")
            kt = persist.tile([128, S], f16, name="kt")
            vt = persist.tile([128, S], f16, name="vt")
            # v1 slots: [V_h | ones | zeros] 128 wide per (k tile, head)
            v1 = persist.tile([128, n_kc * 256], f16, name="v1")
            nc.vector.memset(v1[:], 0.0)
            nc.vector.memset(v1[:, 64::128], 1.0)

            # ---- attention helpers (pools open for the prologue too:
            # the prologue emits early attention batches while the Scalar
            # engine would otherwise idle) -------------------------------
            with (
                tc.tile_pool(name="ps_o", bufs=1, space="PSUM") as ps_o,
                tc.tile_pool(name="etp", bufs=30) as etp,
                tc.tile_pool(name="outp", bufs=2) as outp,
            ):
                def emit_evs(evs):
                    for (ppo, ph, pkc, pet, poff) in evs:
                        nc.tensor.matmul(
                            ppo[:],
                            lhsT=v1[:, pkc * 256 + ph * 128:
                                    pkc * 256 + (ph + 1) * 128],
                            rhs=pet[:, poff:poff + 512],
                            start=(pkc == 0),
                            stop=(pkc == n_kc - 1),
                        )

                def emit_norm(po, qc):
                    # ship unnormalized O^T (+ row-sum row) straight out;
                    # the softmax division happens on the host
                    for h in range(2):
                        otT = outp.tile([65, 512], f16, name="otT",
                                        tag=f"otT{h}")
                        nc.vector.tensor_copy(otT[:], po[h][0:65, :])
                        nc.sync.dma_start(
                            outT[h * 65:(h + 1) * 65,
                                 qc * 512:(qc + 1) * 512],
                            otT[:],
                        )

                # Deferred EV batches: (evs, norm_fn).  At least two stay
                # in flight so an emitted EV's exp is always done and never
                # blocks the next scores at the head of the PE queue.  The
                # prologue's early batches pile up here and drain through
                # q-chunks 0-1.
                pend = []

                def drain_one():
                    evs, norm_fn = pend.pop(0)
                    emit_evs(evs)
                    if norm_fn is not None:
                        norm_fn()

                def emit_batch(po, qs, batch, st_ps, et, last):
                    for si, (kc, h) in enumerate(batch):
                        hp = slice(h * 64, (h + 1) * 64)
                        nc.tensor.matmul(
                            st_ps[:, si * 512:(si + 1) * 512],
                            lhsT=kt[hp, kc * 128:(kc + 1) * 128],
                            rhs=qt[hp, qs],
                            start=True,
                            stop=True,
                        )
                    nc.scalar.activation(et[:], st_ps[:], AF.Exp, scale=0.125)
                    pend.append(([(po[h], h, kc, et, si * 512)
                                  for si, (kc, h) in enumerate(batch)],
                                 last))

                po0 = [
                    ps_o.tile([128, 512], f32, name=f"po{h}", tag=f"po{h}")
                    for h in range(2)
                ]

                # ---- prologue: biases, per-chunk K/V/Q projections + V
                # transposes, plus 2 early attention batches per chunk so
                # the Scalar engine exps while the projections stream ----
                with tc.tile_pool(name="pproj", bufs=2, space="PSUM") as pproj:
                    for i, dst in enumerate([bk_sb, bv_sb, bq_sb]):
                        pb = pproj.tile([128, 1], f32, name="pb", tag="pt")
                        nc.tensor.transpose(
                            pb[:], bsb[0:1, i * 128:(i + 1) * 128], ones11[:])
                        nc.vector.tensor_copy(dst[:], pb[:])
                    qs0 = slice(0, 512)
                    for j in range(n_qc):
                        cs = slice(j * 512, (j + 1) * 512)
                        pk = pproj.tile([128, 512], f32, name="pk", tag="pp")
                        for dc in range(n_dc):
                            nc.tensor.matmul(
                                pk[:],
                                lhsT=wk_sb[:, dc * 128:(dc + 1) * 128],
                                rhs=xt[dc][:, cs],
                                start=(dc == 0),
                                stop=(dc == n_dc - 1),
                            )
                        nc.vector.tensor_scalar_add(kt[:, cs], pk[:], bk_sb[:])
                        pv = pproj.tile([128, 512], f32, name="pv", tag="pp")
                        for dc in range(n_dc):
                            nc.tensor.matmul(
                                pv[:],
                                lhsT=wv_sb[:, dc * 128:(dc + 1) * 128],
                                rhs=xt[dc][:, cs],
                                start=(dc == 0),
                                stop=(dc == n_dc - 1),
                            )
                        nc.vector.tensor_scalar_add(vt[:, cs], pv[:], bv_sb[:])
                        pq = pproj.tile([128, 512], f32, name="pq", tag="pp")
                        for dc in range(n_dc):
                            nc.tensor.matmul(
                                pq[:],
                                lhsT=wq_sb[:, dc * 128:(dc + 1) * 128],
                                rhs=xt[dc][:, cs],
                                start=(dc == 0),
                                stop=(dc == n_dc - 1),
                            )
                        nc.scalar.activation(qt[:, cs], pq[:], AF.Identity,
                                             bias=bq_sb[:])
                        for t in range(4):
                            k = j * 4 + t
                            ptp = pproj.tile([128, 128], f16, name="ptp",
                                             tag="pt")
                            nc.tensor.transpose(
                                ptp[:],
                                vt[:, j * 512 + t * 128: j * 512 + (t + 1) * 128],
                                ident[:],
                            )
                            nc.vector.tensor_copy(
                                v1[:, k * 256:k * 256 + 64], ptp[:, 0:64]
                            )
                            nc.vector.tensor_copy(
                                v1[:, k * 256 + 128:k * 256 + 192],
                                ptp[:, 64:128]
                            )
                        for kce in (3 * j, 3 * j + 1, 3 * j + 2):
                            stE = pproj.tile([128, 1024], f32, name="stE",
                                             tag="stE", bufs=1)
                            etE = etp.tile([128, 1024], f16, name="et",
                                           tag="et")
                            emit_batch(po0, qs0, [(kce, 0), (kce, 1)],
                                       stE, etE, None)


                # ---- main attention loop --------------------------------
                with tc.tile_pool(name="ps_st", bufs=2, space="PSUM") as ps_st:
                    for qc in range(n_qc):
                        qs = slice(qc * 512, (qc + 1) * 512)
                        if qc == 0:
                            po = po0
                            slices = [(kc, h) for kc in range(24, n_kc)
                                      for h in range(2)]
                        else:
                            po = [
                                ps_o.tile([128, 512], f32, name=f"po{h}",
                                          tag=f"po{h}")
                                for h in range(2)
                            ]
                            slices = [(kc, h) for kc in range(n_kc)
                                      for h in range(2)]
                        while slices:
                            nsl = min(3, len(slices))
                            w = nsl * 512
                            st_ps = ps_st.tile([128, w], f32, name="st_ps",
                                               tag="st")
                            et = etp.tile([128, w], f16, name="et", tag="et")
                            batch, slices = slices[:nsl], slices[nsl:]
                            lastfn = (
                                (lambda po=po, qc=qc: emit_norm(po, qc))
                                if not slices else None
                            )
                            emit_batch(po, qs, batch, st_ps, et, lastfn)
                            if qc == 0:
                                if len(pend) >= 3:
                                    drain_one()
                            else:
                                while len(pend) >= 3:
                                    drain_one()
                    while pend:
                        drain_one()
    return nc


def _shard_inputs(x, Wq, bq, Wk, bk, Wv, bv):
    """Build the 8 per-core input maps from full inputs."""
    x = np.asarray(x, dtype=np.float32)
    in_maps = []
    for c in range(N_CORES):
        b, pair = c // 4, c % 4
        rows = slice(pair * 128, (pair + 1) * 128)
        wq_s = np.asarray(Wq)[rows, :].astype(np.float32)
        wk_s = np.asarray(Wk)[rows, :].astype(np.float32)
        wv_s = np.asarray(Wv)[rows, :].astype(np.float32)
        bq_s = np.asarray(bq)[rows].astype(np.float32)
        bk_s = np.asarray(bk)[rows].astype(np.float32)
        bv_s = np.asarray(bv)[rows].astype(np.float32)

        in_maps.append({
            "xT": np.ascontiguousarray(x[b].T).astype(np.float16),
            "wqS": np.ascontiguousarray(
                wq_s.reshape(128, 4, 128).transpose(2, 1, 0).reshape(128, 512)
            ).astype(np.float16),
            "wkT": np.ascontiguousarray(wk_s.T).astype(np.float16),
            "wvS": np.ascontiguousarray(
                wv_s.reshape(128, 4, 128).transpose(2, 1, 0).reshape(128, 512)
            ).astype(np.float16),
            "bkvqT": np.concatenate(
                [bk_s, bv_s, bq_s]).reshape(1, 384).astype(np.float32),
        })
    return in_maps


def _gather(results):
    B, S, D = 2, S_FULL, D_MODEL
    out = np.empty((B, S, D), np.float32)
    for c in range(N_CORES):
        b, pair = c // 4, c % 4
        o = results[c]["outT"].astype(np.float32)
        for h in range(2):
            num = o[h * 65:h * 65 + 64]          # [64, S]
            den = o[h * 65 + 64]                 # [S]
            out[b, :, pair * 128 + h * 64: pair * 128 + (h + 1) * 64] = \
                (num / den).T
    return out


def _install_profile_hook():
    """Provide antenv.axon_hooks (missing in this image) so that
    run_bass_kernel_spmd(trace=True) can capture NTFF profiles, using the
    same ctypes path trn_boot.py would have registered."""
    import sys, types, ctypes, contextlib

    if "antenv.axon_hooks" in sys.modules:
        return
    so_path = "/opt/axon/libaxon_pjrt.so"
    mod = types.ModuleType("antenv.axon_hooks")
    state = {"hook": None}
    mod.set_axon_ntff_profile_hook = lambda h: state.__setitem__("hook", h)
    mod.get_axon_ntff_profile_hook = lambda: state["hook"]
    sys.modules["antenv.axon_hooks"] = mod
    try:
        lib = ctypes.CDLL(so_path)
        if not hasattr(lib, "axon_start_nrt_profile"):
            return
        lib.axon_start_nrt_profile.argtypes = [
            ctypes.POINTER(ctypes.c_int64), ctypes.c_size_t]
        lib.axon_start_nrt_profile.restype = ctypes.c_int64
        lib.axon_stop_nrt_profile.argtypes = [ctypes.c_char_p]
        lib.axon_stop_nrt_profile.restype = ctypes.c_int64

        @contextlib.contextmanager
        def _hook(output_dir, device_ids):
            import jax
            jax.devices()
            if device_ids:
                ids = (ctypes.c_int64 * len(device_ids))(*device_ids)
                rc = lib.axon_start_nrt_profile(ids, len(device_ids))
            else:
                rc = lib.axon_start_nrt_profile(None, 0)
            if rc != 0:
                raise RuntimeError(f"axon_start_nrt_profile rc={rc}")
            try:
                yield
            finally:
                n = lib.axon_stop_nrt_profile(str(output_dir).encode())
                print(f"profile: {n} file(s) written to {output_dir}")

        state["hook"] = _hook
    except OSError:
        pass


def kernel(x, Wq, bq, Wk, bk, Wv, bv, trace=False):
    from concourse.bass_utils import run_bass_kernel_spmd

    if trace:
        _install_profile_hook()
    if "nc" not in _cached:
        nc = build_nc(S_FULL)
        nc.finalize()
        _cached["nc"] = nc
    nc = _cached["nc"]
    in_maps = _shard_inputs(x, Wq, bq, Wk, bk, Wv, bv)
    r = run_bass_kernel_spmd(nc, in_maps, list(range(N_CORES)), trace=trace)
    _cached["last_results"] = r
    return _gather(r.results)


# revision 26
# speedup vs baseline: 1.0195x; 1.0195x over previous
"""Multi-head attention (B=2, S=4096, D=512, H=8) on 8 NeuronCores.

Sharding: data-parallel on batch x head-pair-parallel.  Core c handles
batch b = c//4 and heads (2*(c%4), 2*(c%4)+1); the host scatters inputs
and gathers/normalizes outputs.

Per-core kernel (Bass/Tile), fp16 operands, fp32 PSUM accumulate:
  - DMA order tuned for the 8 HW queue groups: wk chunks + x^T block 0
    first, the other weights pre-swizzled host-side into single 1KB-row
    [128,512] DMAs; biases ship as one [1,384] row and are PE-transposed
    to per-partition columns on core.
  - Prologue per 512-wide chunk: K^T/V^T/Q^T projections (weight
    stationary, x^T streaming, bias-add on the otherwise idle Scalar
    engine); V^T tiles are PE-transposed back to natural [k, d] layout
    into 128-wide [V_h | ones | 0] weight slots (the ones column makes
    the E@V matmul emit softmax row sums in PSUM row 64).
  - Attention per 512-wide q chunk / 128-wide k tile:
      S^T tile  = K^T.T @ Q^T   (row-packed K=64 matmul pairs)
      E         = exp(S^T / 8)  (ACT over [128,1536] PSUM chunks, the
                                 1.2GHz-bound critical path: 176
                                 back-to-back EXPs ~= 250us)
      O^T      += V1.T @ E      (PSUM row 64 accumulates row sums)
    E@V batches are emitted TWO batches late so a deferred EV's exp is
    always complete -- the in-order PE queue then never stalls the next
    batch's S^T matmuls behind an exp-blocked EV.
  - Output: unnormalized O^T halves + row-sum rows DMA out as fp16
    [130, S]; the softmax division happens on the host in _gather.

Measured on the 8 axon trn2 cores: ~292us HW exec (329us baseline),
rel err ~7.6e-4.  At full clock the exp stream is gapless; occasional
runs see a ~20% lower DVFS clock (~358us) independent of the kernel.
"""

import numpy as np

N_CORES = 8
S_FULL = 4096
D_MODEL = 512
HEAD = 64

_cached = {}


def build_nc(S=S_FULL):
    import concourse.bass as bass
    from concourse import bacc
    import concourse.mybir as mybir
    import concourse.tile as tile
    from concourse.masks import make_identity
    f32 = mybir.dt.float32
    f16 = mybir.dt.float16
    AF = mybir.ActivationFunctionType

    D = D_MODEL
    n_qc = S // 512     # 512-wide query chunks
    n_kc = S // 128     # 128-wide key tiles
    n_dc = D // 128     # 128-wide contraction chunks of D

    nc = bacc.Bacc()

    xT = nc.dram_tensor("xT", [D, S], f16, kind="ExternalInput")
    wqS = nc.dram_tensor("wqS", [128, 512], f16, kind="ExternalInput")
    wkT = nc.dram_tensor("wkT", [D, 128], f16, kind="ExternalInput")
    wvS = nc.dram_tensor("wvS", [128, 512], f16, kind="ExternalInput")
    bkvqT = nc.dram_tensor("bkvqT", [1, 384], f32, kind="ExternalInput")
    outT = nc.dram_tensor("outT", [130, S], f16, kind="ExternalOutput")

    with tile.TileContext(nc) as tc:
        with (
            tc.tile_pool(name="consts", bufs=1) as consts,
            tc.tile_pool(name="persist", bufs=1) as persist,
        ):
            ident = consts.tile([128, 128], f16, name="ident")
            make_identity(nc, ident)
            # Preload the exp table set while DMAs run.
            warm = consts.tile([128, 1], f16, name="warm")
            nc.scalar.activation(warm[:], ident[:, 0:1], AF.Exp, scale=0.125)

            wq_sb = consts.tile([128, n_dc * 128], f16, name="wq_sb")
            wk_sb = consts.tile([128, n_dc * 128], f16, name="wk_sb")
            wv_sb = consts.tile([128, n_dc * 128], f16, name="wv_sb")
            bq_sb = consts.tile([128, 1], f32, name="bq_sb")
            bk_sb = consts.tile([128, 1], f32, name="bk_sb")
            bv_sb = consts.tile([128, 1], f32, name="bv_sb")
            bsb = consts.tile([1, 384], f32, name="bsb")
            ones11 = consts.tile([1, 1], f32, name="ones11")
            nc.vector.memset(ones11[:], 1.0)
            xt = [persist.tile([128, S], f16, name=f"xt{i}") for i in range(n_dc)]
            # bias rows (3 descriptors) + x block 0 + wk first: they land at
            # the head of the 8 HW queue groups (round-robin by emission
            # order) and run in parallel; the rest follows in later rounds.
            # wk split in 4 (earliest arrival), x block 0 in halves; the
            # other weights ship pre-swizzled as single 1KB-row DMAs
            for dc in range(n_dc):
                r = slice(dc * 128, (dc + 1) * 128)
                nc.sync.dma_start(wk_sb[:, dc * 128:(dc + 1) * 128], wkT[r, :])
            for dc in range(n_dc):
                for hh in range(2):
                    nc.sync.dma_start(
                        xt[dc][:, hh * 256:(hh + 1) * 256],
                        xT[dc * 128:(dc + 1) * 128, hh * 256:(hh + 1) * 256],
                    )
            nc.sync.dma_start(bsb[:], bkvqT[:, :])
            nc.sync.dma_start(wv_sb[:], wvS[:, :])
            nc.sync.dma_start(wq_sb[:], wqS[:, :])
            for j in range(1, n_qc):
                cs = slice(j * 512, (j + 1) * 512)
                for dc in range(n_dc):
                    nc.sync.dma_start(xt[dc][:, cs],
                                      xT[dc * 128:(dc + 1) * 128, cs])

            qt = persist.tile([128, S], f16, name="qt")
            kt = persist.tile([128, S], f16, name="kt")
            vt = persist.tile([128, S], f16, name="vt")
            # v1 slots: [V_h | ones | zeros] 128 wide per (k tile, head)
            v1 = persist.tile([128, n_kc * 256], f16, name="v1")
            nc.vector.memset(v1[:], 0.0)
            nc.vector.memset(v1[:, 64::128], 1.0)

            # ---- attention helpers (pools open for the prologue too:
            # the prologue emits early attention batches while the Scalar
            # engine would otherwise idle) -------------------------------
            with (
                tc.tile_pool(name="ps_o", bufs=1, space="PSUM") as ps_o,
                tc.tile_pool(name="etp", bufs=20) as etp,
                tc.tile_pool(name="outp", bufs=2) as outp,
            ):
                def emit_evs(evs):
                    for (ppo, ph, pkc, pet, poff) in evs:
                        nc.tensor.matmul(
                            ppo[:],
                            lhsT=v1[:, pkc * 256 + ph * 128:
                                    pkc * 256 + (ph + 1) * 128],
                            rhs=pet[:, poff:poff + 512],
                            start=(pkc == 0),
                            stop=(pkc == n_kc - 1),
                        )

                def emit_norm(po, qc):
                    # ship unnormalized O^T (+ row-sum row) straight out;
                    # the softmax division happens on the host
                    for h in range(2):
                        otT = outp.tile([65, 512], f16, name="otT",
                                        tag=f"otT{h}")
                        nc.vector.tensor_copy(otT[:], po[h][0:65, :])
                        nc.sync.dma_start(
                            outT[h * 65:(h + 1) * 65,
                                 qc * 512:(qc + 1) * 512],
                            otT[:],
                        )

                # Deferred EV batches: (evs, norm_fn).  At least two stay
                # in flight so an emitted EV's exp is always done and never
                # blocks the next scores at the head of the PE queue.  The
                # prologue's early batches pile up here and drain through
                # q-chunks 0-1.
                pend = []

                def drain_one():
                    evs, norm_fn = pend.pop(0)
                    emit_evs(evs)
                    if norm_fn is not None:
                        norm_fn()

                def emit_batch(po, qs, batch, st_ps, et, last):
                    for si, (kc, h) in enumerate(batch):
                        hp = slice(h * 64, (h + 1) * 64)
                        nc.tensor.matmul(
                            st_ps[:, si * 512:(si + 1) * 512],
                            lhsT=kt[hp, kc * 128:(kc + 1) * 128],
                            rhs=qt[hp, qs],
                            start=True,
                            stop=True,
                        )
                    nc.scalar.activation(et[:], st_ps[:], AF.Exp, scale=0.125)
                    pend.append(([(po[h], h, kc, et, si * 512)
                                  for si, (kc, h) in enumerate(batch)],
                                 last))

                po0 = [
                    ps_o.tile([128, 512], f32, name=f"po{h}", tag=f"po{h}")
                    for h in range(2)
                ]

                # ---- prologue: biases, per-chunk K/V/Q projections + V
                # transposes, plus 2 early attention batches per chunk so
                # the Scalar engine exps while the projections stream ----
                with tc.tile_pool(name="pproj", bufs=2, space="PSUM") as pproj:
                    for i, dst in enumerate([bk_sb, bv_sb, bq_sb]):
                        pb = pproj.tile([128, 1], f32, name="pb", tag="pt")
                        nc.tensor.transpose(
                            pb[:], bsb[0:1, i * 128:(i + 1) * 128], ones11[:])
                        nc.vector.tensor_copy(dst[:], pb[:])
                    qs0 = slice(0, 512)
                    for j in range(n_qc):
                        cs = slice(j * 512, (j + 1) * 512)
                        pk = pproj.tile([128, 512], f32, name="pk", tag="pp")
                        for dc in range(n_dc):
                            nc.tensor.matmul(
                                pk[:],
                                lhsT=wk_sb[:, dc * 128:(dc + 1) * 128],
                                rhs=xt[dc][:, cs],
                                start=(dc == 0),
                                stop=(dc == n_dc - 1),
                            )
                        nc.vector.tensor_scalar_add(kt[:, cs], pk[:], bk_sb[:])
                        pv = pproj.tile([128, 512], f32, name="pv", tag="pp")
                        for dc in range(n_dc):
                            nc.tensor.matmul(
                                pv[:],
                                lhsT=wv_sb[:, dc * 128:(dc + 1) * 128],
                                rhs=xt[dc][:, cs],
                                start=(dc == 0),
                                stop=(dc == n_dc - 1),
                            )
                        nc.vector.tensor_scalar_add(vt[:, cs], pv[:], bv_sb[:])
                        pq = pproj.tile([128, 512], f32, name="pq", tag="pp")
                        for dc in range(n_dc):
                            nc.tensor.matmul(
                                pq[:],
                                lhsT=wq_sb[:, dc * 128:(dc + 1) * 128],
                                rhs=xt[dc][:, cs],
                                start=(dc == 0),
                                stop=(dc == n_dc - 1),
                            )
                        nc.scalar.activation(qt[:, cs], pq[:], AF.Identity,
                                             bias=bq_sb[:])
                        for t in range(4):
                            k = j * 4 + t
                            ptp = pproj.tile([128, 128], f16, name="ptp",
                                             tag="pt")
                            nc.tensor.transpose(
                                ptp[:],
                                vt[:, j * 512 + t * 128: j * 512 + (t + 1) * 128],
                                ident[:],
                            )
                            nc.vector.tensor_copy(
                                v1[:, k * 256:k * 256 + 64], ptp[:, 0:64]
                            )
                            nc.vector.tensor_copy(
                                v1[:, k * 256 + 128:k * 256 + 192],
                                ptp[:, 64:128]
                            )
                        for kce in (2 * j, 2 * j + 1):
                            stE = pproj.tile([128, 1024], f32, name="stE",
                                             tag="stE", bufs=1)
                            etE = etp.tile([128, 1024], f16, name="et",
                                           tag="et")
                            emit_batch(po0, qs0, [(kce, 0), (kce, 1)],
                                       stE, etE, None)


                # ---- main attention loop --------------------------------
                with tc.tile_pool(name="ps_st", bufs=2, space="PSUM") as ps_st:
                    for qc in range(n_qc):
                        qs = slice(qc * 512, (qc + 1) * 512)
                        if qc == 0:
                            po = po0
                            slices = [(kc, h) for kc in range(16, n_kc)
                                      for h in range(2)]
                        else:
                            po = [
                                ps_o.tile([128, 512], f32, name=f"po{h}",
                                          tag=f"po{h}")
                                for h in range(2)
                            ]
                            slices = [(kc, h) for kc in range(n_kc)
                                      for h in range(2)]
                        while slices:
                            nsl = min(3, len(slices))
                            w = nsl * 512
                            st_ps = ps_st.tile([128, w], f32, name="st_ps",
                                               tag="st")
                            et = etp.tile([128, w], f16, name="et", tag="et")
                            batch, slices = slices[:nsl], slices[nsl:]
                            lastfn = (
                                (lambda po=po, qc=qc: emit_norm(po, qc))
                                if not slices else None
                            )
                            emit_batch(po, qs, batch, st_ps, et, lastfn)
                            if qc == 0:
                                if len(pend) >= 3:
                                    drain_one()
                            else:
                                while len(pend) >= 3:
                                    drain_one()
                    while pend:
                        drain_one()
    return nc


def _shard_inputs(x, Wq, bq, Wk, bk, Wv, bv):
    """Build the 8 per-core input maps from full inputs."""
    x = np.asarray(x, dtype=np.float32)
    in_maps = []
    for c in range(N_CORES):
        b, pair = c // 4, c % 4
        rows = slice(pair * 128, (pair + 1) * 128)
        wq_s = np.asarray(Wq)[rows, :].astype(np.float32)
        wk_s = np.asarray(Wk)[rows, :].astype(np.float32)
        wv_s = np.asarray(Wv)[rows, :].astype(np.float32)
        bq_s = np.asarray(bq)[rows].astype(np.float32)
        bk_s = np.asarray(bk)[rows].astype(np.float32)
        bv_s = np.asarray(bv)[rows].astype(np.float32)

        in_maps.append({
            "xT": np.ascontiguousarray(x[b].T).astype(np.float16),
            "wqS": np.ascontiguousarray(
                wq_s.reshape(128, 4, 128).transpose(2, 1, 0).reshape(128, 512)
            ).astype(np.float16),
            "wkT": np.ascontiguousarray(wk_s.T).astype(np.float16),
            "wvS": np.ascontiguousarray(
                wv_s.reshape(128, 4, 128).transpose(2, 1, 0).reshape(128, 512)
            ).astype(np.float16),
            "bkvqT": np.concatenate(
                [bk_s, bv_s, bq_s]).reshape(1, 384).astype(np.float32),
        })
    return in_maps


def _gather(results):
    B, S, D = 2, S_FULL, D_MODEL
    out = np.empty((B, S, D), np.float32)
    for c in range(N_CORES):
        b, pair = c // 4, c % 4
        o = results[c]["outT"].astype(np.float32)
        for h in range(2):
            num = o[h * 65:h * 65 + 64]          # [64, S]
            den = o[h * 65 + 64]                 # [S]
            out[b, :, pair * 128 + h * 64: pair * 128 + (h + 1) * 64] = \
                (num / den).T
    return out


def _install_profile_hook():
    """Provide antenv.axon_hooks (missing in this image) so that
    run_bass_kernel_spmd(trace=True) can capture NTFF profiles, using the
    same ctypes path trn_boot.py would have registered."""
    import sys, types, ctypes, contextlib

    if "antenv.axon_hooks" in sys.modules:
        return
    so_path = "/opt/axon/libaxon_pjrt.so"
    mod = types.ModuleType("antenv.axon_hooks")
    state = {"hook": None}
    mod.set_axon_ntff_profile_hook = lambda h: state.__setitem__("hook", h)
    mod.get_axon_ntff_profile_hook = lambda: state["hook"]
    sys.modules["antenv.axon_hooks"] = mod
    try:
        lib = ctypes.CDLL(so_path)
        if not hasattr(lib, "axon_start_nrt_profile"):
            return
        lib.axon_start_nrt_profile.argtypes = [
            ctypes.POINTER(ctypes.c_int64), ctypes.c_size_t]
        lib.axon_start_nrt_profile.restype = ctypes.c_int64
        lib.axon_stop_nrt_profile.argtypes = [ctypes.c_char_p]
        lib.axon_stop_nrt_profile.restype = ctypes.c_int64

        @contextlib.contextmanager
        def _hook(output_dir, device_ids):
            import jax
            jax.devices()
            if device_ids:
                ids = (ctypes.c_int64 * len(device_ids))(*device_ids)
                rc = lib.axon_start_nrt_profile(ids, len(device_ids))
            else:
                rc = lib.axon_start_nrt_profile(None, 0)
            if rc != 0:
                raise RuntimeError(f"axon_start_nrt_profile rc={rc}")
            try:
                yield
            finally:
                n = lib.axon_stop_nrt_profile(str(output_dir).encode())
                print(f"profile: {n} file(s) written to {output_dir}")

        state["hook"] = _hook
    except OSError:
        pass


def kernel(x, Wq, bq, Wk, bk, Wv, bv, trace=False):
    from concourse.bass_utils import run_bass_kernel_spmd

    if trace:
        _install_profile_hook()
    if "nc" not in _cached:
        nc = build_nc(S_FULL)
        nc.finalize()
        _cached["nc"] = nc
    nc = _cached["nc"]
    in_maps = _shard_inputs(x, Wq, bq, Wk, bk, Wv, bv)
    r = run_bass_kernel_spmd(nc, in_maps, list(range(N_CORES)), trace=trace)
    _cached["last_results"] = r
    return _gather(r.results)


# revision 27
# speedup vs baseline: 1.0216x; 1.0020x over previous
"""Multi-head attention (B=2, S=4096, D=512, H=8) on 8 NeuronCores.

Sharding: data-parallel on batch x head-pair-parallel.  Core c handles
batch b = c//4 and heads (2*(c%4), 2*(c%4)+1); the host scatters inputs
and gathers/normalizes outputs.

Per-core kernel (Bass/Tile), fp16 operands, fp32 PSUM accumulate:
  - DMA order tuned for the 8 HW queue groups: wk chunks + x^T block 0
    first, the other weights pre-swizzled host-side into single 1KB-row
    [128,512] DMAs; biases ship as one [1,384] row and are PE-transposed
    to per-partition columns on core.
  - Prologue per 512-wide chunk: K^T/V^T/Q^T projections (weight
    stationary, x^T streaming, bias-add on the otherwise idle Scalar
    engine); V^T tiles are PE-transposed back to natural [k, d] layout
    into 128-wide [V_h | ones | 0] weight slots (the ones column makes
    the E@V matmul emit softmax row sums in PSUM row 64).
  - Attention per 512-wide q chunk / 128-wide k tile:
      S^T tile  = K^T.T @ Q^T   (row-packed K=64 matmul pairs)
      E         = exp(S^T / 8)  (ACT over [128,1536] PSUM chunks, the
                                 1.2GHz-bound critical path: 176
                                 back-to-back EXPs ~= 250us)
      O^T      += V1.T @ E      (PSUM row 64 accumulates row sums)
    E@V batches are emitted TWO batches late so a deferred EV's exp is
    always complete -- the in-order PE queue then never stalls the next
    batch's S^T matmuls behind an exp-blocked EV.
  - Output: unnormalized O^T halves + row-sum rows DMA out as fp16
    [130, S]; the softmax division happens on the host in _gather.

Measured on the 8 axon trn2 cores: ~292us HW exec (329us baseline),
rel err ~7.6e-4.  At full clock the exp stream is gapless; occasional
runs see a ~20% lower DVFS clock (~358us) independent of the kernel.
"""

import numpy as np

N_CORES = 8
S_FULL = 4096
D_MODEL = 512
HEAD = 64

_cached = {}


def build_nc(S=S_FULL):
    import concourse.bass as bass
    from concourse import bacc
    import concourse.mybir as mybir
    import concourse.tile as tile
    from concourse.masks import make_identity
    f32 = mybir.dt.float32
    f16 = mybir.dt.float16
    AF = mybir.ActivationFunctionType

    D = D_MODEL
    n_qc = S // 512     # 512-wide query chunks
    n_kc = S // 128     # 128-wide key tiles
    n_dc = D // 128     # 128-wide contraction chunks of D

    nc = bacc.Bacc()

    xT = nc.dram_tensor("xT", [D, S], f16, kind="ExternalInput")
    wqS = nc.dram_tensor("wqS", [128, 512], f16, kind="ExternalInput")
    wkT = nc.dram_tensor("wkT", [D, 128], f16, kind="ExternalInput")
    wvS = nc.dram_tensor("wvS", [128, 512], f16, kind="ExternalInput")
    bkvqT = nc.dram_tensor("bkvqT", [1, 384], f32, kind="ExternalInput")
    outT = nc.dram_tensor("outT", [130, S], f16, kind="ExternalOutput")

    with tile.TileContext(nc) as tc:
        with (
            tc.tile_pool(name="consts", bufs=1) as consts,
            tc.tile_pool(name="persist", bufs=1) as persist,
        ):
            ident = consts.tile([128, 128], f16, name="ident")
            make_identity(nc, ident)
            # Preload the exp table set while DMAs run.
            warm = consts.tile([128, 1], f16, name="warm")
            nc.scalar.activation(warm[:], ident[:, 0:1], AF.Exp, scale=0.125)

            wq_sb = consts.tile([128, n_dc * 128], f16, name="wq_sb")
            wk_sb = consts.tile([128, n_dc * 128], f16, name="wk_sb")
            wv_sb = consts.tile([128, n_dc * 128], f16, name="wv_sb")
            bq_sb = consts.tile([128, 1], f32, name="bq_sb")
            bk_sb = consts.tile([128, 1], f32, name="bk_sb")
            bv_sb = consts.tile([128, 1], f32, name="bv_sb")
            bsb = consts.tile([1, 384], f32, name="bsb")
            ones11 = consts.tile([1, 1], f32, name="ones11")
            nc.vector.memset(ones11[:], 1.0)
            xt = [persist.tile([128, S], f16, name=f"xt{i}") for i in range(n_dc)]
            # bias rows (3 descriptors) + x block 0 + wk first: they land at
            # the head of the 8 HW queue groups (round-robin by emission
            # order) and run in parallel; the rest follows in later rounds.
            # wk split in 4 (earliest arrival), x block 0 in halves; the
            # other weights ship pre-swizzled as single 1KB-row DMAs
            nc.sync.dma_start(bsb[:], bkvqT[:, :])
            for dc in range(n_dc):
                r = slice(dc * 128, (dc + 1) * 128)
                nc.sync.dma_start(wk_sb[:, dc * 128:(dc + 1) * 128], wkT[r, :])
            for dc in range(n_dc):
                for hh in range(2):
                    nc.sync.dma_start(
                        xt[dc][:, hh * 256:(hh + 1) * 256],
                        xT[dc * 128:(dc + 1) * 128, hh * 256:(hh + 1) * 256],
                    )
            nc.sync.dma_start(wv_sb[:], wvS[:, :])
            nc.sync.dma_start(wq_sb[:], wqS[:, :])
            for j in range(1, n_qc):
                cs = slice(j * 512, (j + 1) * 512)
                for dc in range(n_dc):
                    nc.sync.dma_start(xt[dc][:, cs],
                                      xT[dc * 128:(dc + 1) * 128, cs])

            qt = persist.tile([128, S], f16, name="qt")
            kt = persist.tile([128, S], f16, name="kt")
            vt = persist.tile([128, S], f16, name="vt")
            # v1 slots: [V_h | ones | zeros] 128 wide per (k tile, head)
            v1 = persist.tile([128, n_kc * 256], f16, name="v1")
            nc.vector.memset(v1[:], 0.0)
            nc.vector.memset(v1[:, 64::128], 1.0)

            # ---- attention helpers (pools open for the prologue too:
            # the prologue emits early attention batches while the Scalar
            # engine would otherwise idle) -------------------------------
            with (
                tc.tile_pool(name="ps_o", bufs=1, space="PSUM") as ps_o,
                tc.tile_pool(name="etp", bufs=20) as etp,
                tc.tile_pool(name="outp", bufs=2) as outp,
            ):
                def emit_evs(evs):
                    for (ppo, ph, pkc, pet, poff) in evs:
                        nc.tensor.matmul(
                            ppo[:],
                            lhsT=v1[:, pkc * 256 + ph * 128:
                                    pkc * 256 + (ph + 1) * 128],
                            rhs=pet[:, poff:poff + 512],
                            start=(pkc == 0),
                            stop=(pkc == n_kc - 1),
                        )

                def emit_norm(po, qc):
                    # ship unnormalized O^T (+ row-sum row) straight out;
                    # the softmax division happens on the host
                    for h in range(2):
                        otT = outp.tile([65, 512], f16, name="otT",
                                        tag=f"otT{h}")
                        nc.vector.tensor_copy(otT[:], po[h][0:65, :])
                        nc.sync.dma_start(
                            outT[h * 65:(h + 1) * 65,
                                 qc * 512:(qc + 1) * 512],
                            otT[:],
                        )

                # Deferred EV batches: (evs, norm_fn).  At least two stay
                # in flight so an emitted EV's exp is always done and never
                # blocks the next scores at the head of the PE queue.  The
                # prologue's early batches pile up here and drain through
                # q-chunks 0-1.
                pend = []

                def drain_one():
                    evs, norm_fn = pend.pop(0)
                    emit_evs(evs)
                    if norm_fn is not None:
                        norm_fn()

                def emit_batch(po, qs, batch, st_ps, et, last):
                    for si, (kc, h) in enumerate(batch):
                        hp = slice(h * 64, (h + 1) * 64)
                        nc.tensor.matmul(
                            st_ps[:, si * 512:(si + 1) * 512],
                            lhsT=kt[hp, kc * 128:(kc + 1) * 128],
                            rhs=qt[hp, qs],
                            start=True,
                            stop=True,
                        )
                    nc.scalar.activation(et[:], st_ps[:], AF.Exp, scale=0.125)
                    pend.append(([(po[h], h, kc, et, si * 512)
                                  for si, (kc, h) in enumerate(batch)],
                                 last))

                po0 = [
                    ps_o.tile([128, 512], f32, name=f"po{h}", tag=f"po{h}")
                    for h in range(2)
                ]

                # ---- prologue: biases, per-chunk K/V/Q projections + V
                # transposes, plus 2 early attention batches per chunk so
                # the Scalar engine exps while the projections stream ----
                with tc.tile_pool(name="pproj", bufs=2, space="PSUM") as pproj:
                    for i, dst in enumerate([bk_sb, bv_sb, bq_sb]):
                        pb = pproj.tile([128, 1], f32, name="pb", tag="pt")
                        nc.tensor.transpose(
                            pb[:], bsb[0:1, i * 128:(i + 1) * 128], ones11[:])
                        nc.vector.tensor_copy(dst[:], pb[:])
                    qs0 = slice(0, 512)
                    for j in range(n_qc):
                        cs = slice(j * 512, (j + 1) * 512)
                        pk = pproj.tile([128, 512], f32, name="pk", tag="pp")
                        for dc in range(n_dc):
                            nc.tensor.matmul(
                                pk[:],
                                lhsT=wk_sb[:, dc * 128:(dc + 1) * 128],
                                rhs=xt[dc][:, cs],
                                start=(dc == 0),
                                stop=(dc == n_dc - 1),
                            )
                        nc.vector.tensor_scalar_add(kt[:, cs], pk[:], bk_sb[:])
                        pv = pproj.tile([128, 512], f32, name="pv", tag="pp")
                        for dc in range(n_dc):
                            nc.tensor.matmul(
                                pv[:],
                                lhsT=wv_sb[:, dc * 128:(dc + 1) * 128],
                                rhs=xt[dc][:, cs],
                                start=(dc == 0),
                                stop=(dc == n_dc - 1),
                            )
                        nc.vector.tensor_scalar_add(vt[:, cs], pv[:], bv_sb[:])
                        pq = pproj.tile([128, 512], f32, name="pq", tag="pp")
                        for dc in range(n_dc):
                            nc.tensor.matmul(
                                pq[:],
                                lhsT=wq_sb[:, dc * 128:(dc + 1) * 128],
                                rhs=xt[dc][:, cs],
                                start=(dc == 0),
                                stop=(dc == n_dc - 1),
                            )
                        if j == 0:
                            nc.vector.tensor_scalar_add(qt[:, cs], pq[:],
                                                        bq_sb[:])
                        else:
                            nc.scalar.activation(qt[:, cs], pq[:], AF.Identity,
                                                 bias=bq_sb[:])
                        for t in range(4):
                            k = j * 4 + t
                            ptp = pproj.tile([128, 128], f16, name="ptp",
                                             tag="pt")
                            nc.tensor.transpose(
                                ptp[:],
                                vt[:, j * 512 + t * 128: j * 512 + (t + 1) * 128],
                                ident[:],
                            )
                            nc.vector.tensor_copy(
                                v1[:, k * 256:k * 256 + 64], ptp[:, 0:64]
                            )
                            nc.vector.tensor_copy(
                                v1[:, k * 256 + 128:k * 256 + 192],
                                ptp[:, 64:128]
                            )
                        for kce in (2 * j, 2 * j + 1):
                            stE = pproj.tile([128, 1024], f32, name="stE",
                                             tag="stE", bufs=1)
                            etE = etp.tile([128, 1024], f16, name="et",
                                           tag="et")
                            emit_batch(po0, qs0, [(kce, 0), (kce, 1)],
                                       stE, etE, None)


                # ---- main attention loop --------------------------------
                with tc.tile_pool(name="ps_st", bufs=2, space="PSUM") as ps_st:
                    for qc in range(n_qc):
                        qs = slice(qc * 512, (qc + 1) * 512)
                        if qc == 0:
                            po = po0
                            slices = [(kc, h) for kc in range(16, n_kc)
                                      for h in range(2)]
                        else:
                            po = [
                                ps_o.tile([128, 512], f32, name=f"po{h}",
                                          tag=f"po{h}")
                                for h in range(2)
                            ]
                            slices = [(kc, h) for kc in range(n_kc)
                                      for h in range(2)]
                        while slices:
                            nsl = min(3, len(slices))
                            w = nsl * 512
                            st_ps = ps_st.tile([128, w], f32, name="st_ps",
                                               tag="st")
                            et = etp.tile([128, w], f16, name="et", tag="et")
                            batch, slices = slices[:nsl], slices[nsl:]
                            lastfn = (
                                (lambda po=po, qc=qc: emit_norm(po, qc))
                                if not slices else None
                            )
                            emit_batch(po, qs, batch, st_ps, et, lastfn)
                            if qc == 0:
                                if len(pend) >= 3:
                                    drain_one()
                            else:
                                while len(pend) >= 3:
                                    drain_one()
                    while pend:
                        drain_one()
    return nc


def _shard_inputs(x, Wq, bq, Wk, bk, Wv, bv):
    """Build the 8 per-core input maps from full inputs."""
    x = np.asarray(x, dtype=np.float32)
    in_maps = []
    for c in range(N_CORES):
        b, pair = c // 4, c % 4
        rows = slice(pair * 128, (pair + 1) * 128)
        wq_s = np.asarray(Wq)[rows, :].astype(np.float32)
        wk_s = np.asarray(Wk)[rows, :].astype(np.float32)
        wv_s = np.asarray(Wv)[rows, :].astype(np.float32)
        bq_s = np.asarray(bq)[rows].astype(np.float32)
        bk_s = np.asarray(bk)[rows].astype(np.float32)
        bv_s = np.asarray(bv)[rows].astype(np.float32)

        in_maps.append({
            "xT": np.ascontiguousarray(x[b].T).astype(np.float16),
            "wqS": np.ascontiguousarray(
                wq_s.reshape(128, 4, 128).transpose(2, 1, 0).reshape(128, 512)
            ).astype(np.float16),
            "wkT": np.ascontiguousarray(wk_s.T).astype(np.float16),
            "wvS": np.ascontiguousarray(
                wv_s.reshape(128, 4, 128).transpose(2, 1, 0).reshape(128, 512)
            ).astype(np.float16),
            "bkvqT": np.concatenate(
                [bk_s, bv_s, bq_s]).reshape(1, 384).astype(np.float32),
        })
    return in_maps


def _gather(results):
    B, S, D = 2, S_FULL, D_MODEL
    out = np.empty((B, S, D), np.float32)
    for c in range(N_CORES):
        b, pair = c // 4, c % 4
        o = results[c]["outT"].astype(np.float32)
        for h in range(2):
            num = o[h * 65:h * 65 + 64]          # [64, S]
            den = o[h * 65 + 64]                 # [S]
            out[b, :, pair * 128 + h * 64: pair * 128 + (h + 1) * 64] = \
                (num / den).T
    return out


def _install_profile_hook():
    """Provide antenv.axon_hooks (missing in this image) so that
    run_bass_kernel_spmd(trace=True) can capture NTFF profiles, using the
    same ctypes path trn_boot.py would have registered."""
    import sys, types, ctypes, contextlib

    if "antenv.axon_hooks" in sys.modules:
        return
    so_path = "/opt/axon/libaxon_pjrt.so"
    mod = types.ModuleType("antenv.axon_hooks")
    state = {"hook": None}
    mod.set_axon_ntff_profile_hook = lambda h: state.__setitem__("hook", h)
    mod.get_axon_ntff_profile_hook = lambda: state["hook"]
    sys.modules["antenv.axon_hooks"] = mod
    try:
        lib = ctypes.CDLL(so_path)
        if not hasattr(lib, "axon_start_nrt_profile"):
            return
        lib.axon_start_nrt_profile.argtypes = [
            ctypes.POINTER(ctypes.c_int64), ctypes.c_size_t]
        lib.axon_start_nrt_profile.restype = ctypes.c_int64
        lib.axon_stop_nrt_profile.argtypes = [ctypes.c_char_p]
        lib.axon_stop_nrt_profile.restype = ctypes.c_int64

        @contextlib.contextmanager
        def _hook(output_dir, device_ids):
            import jax
            jax.devices()
            if device_ids:
                ids = (ctypes.c_int64 * len(device_ids))(*device_ids)
                rc = lib.axon_start_nrt_profile(ids, len(device_ids))
            else:
                rc = lib.axon_start_nrt_profile(None, 0)
            if rc != 0:
                raise RuntimeError(f"axon_start_nrt_profile rc={rc}")
            try:
                yield
            finally:
                n = lib.axon_stop_nrt_profile(str(output_dir).encode())
                print(f"profile: {n} file(s) written to {output_dir}")

        state["hook"] = _hook
    except OSError:
        pass


def kernel(x, Wq, bq, Wk, bk, Wv, bv, trace=False):
    from concourse.bass_utils import run_bass_kernel_spmd

    if trace:
        _install_profile_hook()
    if "nc" not in _cached:
        nc = build_nc(S_FULL)
        nc.finalize()
        _cached["nc"] = nc
    nc = _cached["nc"]
    in_maps = _shard_inputs(x, Wq, bq, Wk, bk, Wv, bv)
    r = run_bass_kernel_spmd(nc, in_maps, list(range(N_CORES)), trace=trace)
    _cached["last_results"] = r
    return _gather(r.results)
